# revision 18
# baseline (speedup 1.0000x reference)
"""DPLR SSM block kernel for Trainium2, 8 NeuronCores.

Math:  out = h @ (diag(a_diag) + p q^T).T + x @ b_mat          (B=64, H=8192, R=4)
           = h * a_diag  +  (h @ q) @ p^T  +  x @ b_mat

The dense (H,H) DPLR matrix is never materialized.  Sharding: b_mat columns
(= output features) split 8 ways; core c computes out[:, c*1024:(c+1)*1024].

HBM-bound on streaming b_mat; the correctness budget (rel_err < 2e-2) is
spent on quantization:
  - b as fp8 e3m4 (1 byte/elem), global scale S with max|S*b| ~ 15.49 (just
    under e3m4 max finite).  The descale is folded into x on the host
    (xq = x/S as fp16), so the device never rescales.
  - h/q/p feeding the tiny rank-4 + diagonal terms ride as e3m4 too, with
    pre-scales whose product folds into the one hq PSUM->SBUF copy.
  - output stored as fp16, upcast on host.
Measured end-to-end rel_fro error: ~1.21e-2 (gate 2e-2).

Per-core HBM traffic ~9.7 MB (vs 36.6 MB for the baseline split-bf16
kernel): b 8 MB fp8 + x 1 MB fp16 + h 0.5 MB fp8 + ~0.2 MB small aux +
0.125 MB out.  The modeled DMA roofline (360 GB/s, all queues serialized
through one DMA-engine pool) makes this a ~28.5 us stream; PE work is
~16 us and hides under it.  TimelineSim end-to-end: 33837 ns (~2 us fixed
startup + packed 28.6 us stream + 3.3 us latency tail), vs 117827 ns for
the previous split-bf16 kernel.

PE layout: batch (64) on the moving operand, b column-blocks (128 wide)
stationary -> 64 moving rows per (chunk, group) matmul.  b streams
GROUP-MAJOR (all 64 k-chunks of a 128-column group consecutively), so group
g's PSUM accumulation finishes at (g+1)/8 of the stream; its epilogue and
output ride mid-stream and only the last group's epilogue is on the tail.
The rank-4 term goes through a second PSUM bank and is pre-folded with the
diagonal term into out_sb mid-stream:
    fold (mid-stream):  out_sb[g] = hd_g * ad_g + psR_g      [DVE, fused]
    epilogue per group: out_sb[g] += psA_g                   [DVE]

Tail: the final two groups' 32 KB store goes through a SWDGE
prepare/trigger pair -- descriptors are written mid-stream on the idle
GPSIMD queue and the tail pays a ~40ns trigger + transfer instead of the
~1.3us HWDGE issue chain of a regular dma_start.  The scatter-add target
region of `o` is zeroed by a small DMA up front.

Per core c (j0 = c*1024, groups g of 128 columns):
  psA[:,g,:] (128,64) = sum_ko  bS[g,ko]^T(128x128) . xq[ko](128x64) [PE f8xf16]
  pshq (4,64)         = sum_ko  q8[ko]^T(128x4) . h8[ko](128x64)     [PE f8]
                        (h8/q8 packed in one fp8 aux tensor hq8)
  hqt (4,64)          = pshq / (HT*QK*PT scales)                     [DVE]
  psR[:,g,:] (128,64) = pt[g]^T(4x128) . hqt(4x64)                   [PE f16]
"""

import ml_dtypes
import numpy as np

import concourse.bass as bass
import concourse.mybir as mybir
from concourse import bacc
from concourse.bass_utils import run_bass_kernel_spmd
from concourse.tile import TileContext

H = 8192
R = 4
B = 64
NCORES = 8
JS = H // NCORES  # 1024 output columns per core
P = 128
G = JS // P  # 8 column groups of 128 per core
GP = G // 2  # groups pair up in the output layout (256B scatter rows)
KO = H // P  # 64 k-chunks

F32 = mybir.dt.float32
F16 = mybir.dt.float16
F8 = mybir.dt.float8e3
I16 = mybir.dt.int16

NP_F16 = np.float16
NP_F8 = ml_dtypes.float8_e3m4
E3M4_MAX_SAFE = 15.49  # just under e3m4 max finite (15.5); no overflow to inf
HT_SCALE = 2.0  # pre-scale on h for the hq matmul (fewer e3m4 subnormals)
QK_SCALE = 512.0  # pre-scale on q (e3m4 subnormal floor is 2^-6)
PT_SCALE = 512.0  # pre-scale on p for its e3m4 carry

# b-tile taper in k-chunks per group: big tiles for full-rate streaming,
# small tail tiles on the LAST group so the final-byte -> final-matmul ->
# epilogue chain is short.
TILES_MID = [32, 32]
TILES_LAST = [32, 20, 8, 4]
MAXKT = 32


def _build_nc(bufs: int = 12) -> bass.Bass:
    nc = bacc.Bacc("TRN2", target_bir_lowering=False, debug=False, num_devices=NCORES)

    xq = nc.dram_tensor("xq", (P, KO, B), F16, kind="ExternalInput")
    hq8 = nc.dram_tensor("hq8", (P, KO, B + R), F8, kind="ExternalInput")
    pt = nc.dram_tensor("pt", (R, JS), F8, kind="ExternalInput")
    bm = nc.dram_tensor("bm", (G, P, KO, P), F8, kind="ExternalInput")
    hd = nc.dram_tensor("hd", (P, G, B), F8, kind="ExternalInput")
    ad = nc.dram_tensor("ad", (P, G), F32, kind="ExternalInput")
    ix = nc.dram_tensor("ix", (P, G), I16, kind="ExternalInput")
    o = nc.dram_tensor("o", (P, GP, 2 * B), F16, kind="ExternalOutput")

    sc_sem = nc.alloc_semaphore("sc7")

    with TileContext(nc) as tc:
        with (
            tc.tile_pool(name="persist", bufs=1) as persist,
            tc.tile_pool(name="bpool", bufs=bufs) as bpool,
            tc.tile_pool(name="psum", bufs=1, space="PSUM") as psum_pool,
        ):
            xq_sb = persist.tile([P, KO, B], F16)
            hq8_sb = persist.tile([P, KO, B + R], F8)
            pt_sb = persist.tile([R, JS], F8)
            hd_sb = persist.tile([P, G, B], F8)
            ad_sb = persist.tile([P, G], F32)
            ix_sb = persist.tile([P, G], I16)
            z_sb = persist.tile([P, 1, 2 * B], F16)
            out_sb = persist.tile([P, GP, 2 * B], F16)
            hqt_sb = persist.tile([R, B], F16)

            psA = psum_pool.tile([P, G, B], F32)  # main accum, one PSUM bank
            psR = psum_pool.tile([P, G, B], F32)  # rank-4 term, one PSUM bank
            pshq = psum_pool.tile([R, B], F32)

            def oap(g):
                # group g's [128, 64] slice of the paired output layout
                return out_sb[:, g // 2, (g % 2) * B : (g % 2 + 1) * B]

            # Aux stream on the Activation HWDGE ring.  xq first (the first
            # main matmuls need it), everything else behind it; b tiles ride
            # the SP ring concurrently and interleave at the DMA engines.
            nc.scalar.dma_start(out=xq_sb[:], in_=xq[:, :, :])
            nc.scalar.dma_start(out=hq8_sb[:], in_=hq8[:, :, :])
            nc.scalar.dma_start(out=hd_sb[:], in_=hd[:, :, :])
            nc.scalar.dma_start(out=ad_sb[:], in_=ad[:, :])
            nc.scalar.dma_start(out=pt_sb[:], in_=pt[:, :])
            nc.scalar.dma_start(out=ix_sb[:], in_=ix[:, :])
            # Zero the scatter-add target region (groups 6-7 of o).
            nc.vector.memset(z_sb[:], 0.0)
            nc.scalar.dma_start(out=o[:, GP - 1 : GP], in_=z_sb[:])

            n_main = KO * G
            im = 0
            for g in range(G):
                tiles = TILES_LAST if g == G - 1 else TILES_MID
                ko = 0
                for kt in tiles:
                    bfull = bpool.tile([P, MAXKT, P], F8, name="btile")
                    btile = bfull[:, :kt]
                    nc.sync.dma_start(out=btile[:], in_=bm[g, :, ko : ko + kt])
                    for k4 in range(kt):
                        nc.tensor.matmul(
                            psA[:, g],
                            btile[:, k4],
                            xq_sb[:, ko],
                            start=(im == 0),
                            stop=(im == n_main - 1),
                        )
                        im += 1
                        ko += 1

                if g == 1:
                    # After two groups of mains the PE has plenty of slack
                    # and ht/qk/pt have landed: fold the whole DPLR low-rank
                    # path here, mid-stream.
                    for ko_q in range(KO):
                        nc.tensor.matmul(
                            pshq[:],
                            hq8_sb[:, ko_q, B : B + R],
                            hq8_sb[:, ko_q, 0:B],
                            start=(ko_q == 0),
                            stop=(ko_q == KO - 1),
                        )
                    nc.vector.tensor_scalar_mul(
                        hqt_sb[:], pshq[:], 1.0 / (HT_SCALE * QK_SCALE * PT_SCALE)
                    )
                    for gr in range(G):
                        nc.tensor.matmul(
                            psR[:, gr],
                            pt_sb[:, gr * P : (gr + 1) * P],
                            hqt_sb[:],
                            start=(gr == 0),
                            stop=(gr == G - 1),
                        )
                    for gr in range(G):
                        nc.vector.scalar_tensor_tensor(
                            out=oap(gr),
                            in0=hd_sb[:, gr],
                            scalar=ad_sb[:, gr : gr + 1],
                            in1=psR[:, gr],
                            op0=mybir.AluOpType.mult,
                            op1=mybir.AluOpType.add,
                        )
                    # SWDGE descriptors for the final groups' store; the DMA
                    # itself fires from trigger_dma at the tail.  Data deps
                    # (the epilogue adds into out_sb[:, 3]) sit on the
                    # trigger, not the prep.
                    nc.gpsimd.dma_scatter_add(
                        o[:, GP - 1 : GP, :],
                        out_sb[:, GP - 1 : GP, :],
                        ix_sb[:],
                        P,
                        P,
                        2 * B,
                        elem_step=G * B,
                        prepare_only=True,
                        sem=sc_sem,
                    )
                    # groups 0 and 1 finished their mains before the fold:
                    # emit their epilogue adds now.
                    for gr in (0, 1):
                        nc.vector.tensor_add(
                            out=oap(gr), in0=oap(gr), in1=psA[:, gr]
                        )
                elif g > 1:
                    nc.vector.tensor_add(out=oap(g), in0=oap(g), in1=psA[:, g])
                # Outputs for groups 0-5 ride in 2 batched mid-stream DMAs;
                # groups 6-7 go through the prepared scatter at the tail.
                if g == 3:
                    nc.scalar.dma_start(out=o[:, 0:2], in_=out_sb[:, 0:2])
                elif g == G - 2:
                    nc.scalar.dma_start(out=o[:, 2:3], in_=out_sb[:, 2:3])
                elif g == G - 1:
                    nc.gpsimd.trigger_dma(count=None)

    nc.finalize()

    # Tile assigns the scatter prep a DMASW completion lane and makes the
    # exit drain wait on that lane's semaphore, but leaves the user sem in
    # the descriptor's completion slot (on_update[0]).  Retarget the
    # completion update at the lane sem the drain actually waits on (this is
    # exactly what Tile wires up for non-prepared SWDGE DMAs).
    fn = nc.m.functions[0]
    lane_wait = None
    prep = None
    for blk in list(fn.blocks):
        for inst in list(blk.instructions):
            si = inst.sync_info
            if si is None:
                continue
            for w in si.on_wait:
                if w.ant_name and w.ant_name.startswith("DMASW"):
                    lane_wait = w
            if type(inst).__name__ == "InstDMAScatterAddAnt":
                prep = inst
    assert prep is not None and lane_wait is not None
    upd = prep.sync_info.on_update[0]
    assert upd.ant_name == "sc7", upd
    upd.id = lane_wait.id
    upd.ant_name = lane_wait.ant_name
    return nc


_NC_CACHE = None


def _get_nc() -> bass.Bass:
    global _NC_CACHE
    if _NC_CACHE is None:
        _NC_CACHE = _build_nc()
    return _NC_CACHE


def _in_maps(h, x, a_diag, p_vec, q_vec, b_mat):
    bmax = float(np.abs(b_mat).max())
    S = E3M4_MAX_SAFE / bmax if bmax > 0 else 1.0

    # Replicated inputs, k-on-partitions chunk layout.
    # xq[ki, ko, b] = x[b, ko*128 + ki] / S   (b descale folded into x)
    xq = np.ascontiguousarray(
        (x / S).astype(NP_F16).reshape(B, KO, P).transpose(2, 1, 0)
    )
    hq8 = np.empty((P, KO, B + R), dtype=NP_F8)
    hq8[:, :, 0:B] = (h * HT_SCALE).astype(NP_F8).reshape(B, KO, P).transpose(2, 1, 0)
    hq8[:, :, B : B + R] = (
        (q_vec * QK_SCALE).astype(NP_F8).reshape(KO, P, R).transpose(1, 0, 2)
    )

    # Scatter identity indices, wrapped in 16 partitions and replicated
    # across the rest: idx i decodes from [i % 16, i // 16].
    ii = np.arange(P)
    ixw = np.ascontiguousarray(
        ((ii[:, None] % 16) + 16 * np.arange(G)[None, :]).astype(np.int16)
    )

    # bq[ko, ki, c, g, j] = S*b[ko*128+ki, (c*8+g)*128+j], quantized once.
    bq = (b_mat * S).astype(NP_F8).reshape(KO, P, NCORES, G, P)

    in_maps = []
    for c in range(NCORES):
        j0 = c * JS
        bc = np.ascontiguousarray(bq[:, :, c].transpose(2, 1, 0, 3))  # (G,P,KO,P)
        # hd[j, g, b] = h[b, j0 + g*128 + j];  ad[j, g] = a_diag[j0 + g*128 + j]
        hdc = np.ascontiguousarray(
            h[:, j0 : j0 + JS].astype(NP_F8).reshape(B, G, P).transpose(2, 1, 0)
        )
        adc = np.ascontiguousarray(
            a_diag[j0 : j0 + JS].reshape(G, P).T.astype(np.float32)
        )
        in_maps.append(
            {
                "ix": ixw,
                "xq": xq,
                "hq8": hq8,
                "pt": np.ascontiguousarray(
                    (p_vec[j0 : j0 + JS, :] * PT_SCALE).T.astype(NP_F8)
                ),
                "bm": bc,
                "hd": hdc,
                "ad": adc,
            }
        )
    return in_maps


def kernel(h, x, a_diag, p_vec, q_vec, b_mat) -> np.ndarray:
    h = np.ascontiguousarray(np.asarray(h, dtype=np.float32))
    x = np.ascontiguousarray(np.asarray(x, dtype=np.float32))
    a_diag = np.asarray(a_diag, dtype=np.float32)
    p_vec = np.asarray(p_vec, dtype=np.float32)
    q_vec = np.asarray(q_vec, dtype=np.float32)
    b_mat = np.asarray(b_mat, dtype=np.float32)

    nc = _get_nc()
    res = run_bass_kernel_spmd(
        nc, _in_maps(h, x, a_diag, p_vec, q_vec, b_mat), core_ids=list(range(NCORES))
    )
    # o[j, gg, gh*64 + b] -> out[b, (2*gg + gh)*128 + j]
    outs = [
        np.asarray(r["o"])
        .astype(np.float32)
        .reshape(P, GP, 2, B)
        .transpose(3, 1, 2, 0)
        .reshape(B, JS)
        for r in res.results
    ]
    return np.concatenate(outs, axis=1)
